# revision 11
# baseline (speedup 1.0000x reference)
"""DigitCapsuleLayer forward (2 routing iterations) on 8 Trainium2 cores.

Pure data-parallel: batch 256 is split 32-per-core. All heavy contractions
run on the PE array in bf16 with f32 PSUM accumulation; routing math is
restructured so u_hat [B,2,6912,16] is never materialized:

  S[b,je]    = sum_m Wf[m,je] * x[m,b]          (m = (n,d) flattened, 55296)
  v1         = squash(0.5*S)
  g[m,b]     = sum_je Wf[m,je] * vtil[je,b]     (vtil = [v1_j0, -v1_j1])
  Delta[n,b] = sum_d g[(n,d),b] * x[(n,d),b]    (block-diag ones matmul)
  c0         = sigmoid(Delta) broadcast over d  (replication matmul)
  y0         = c0 * x
  A[b,je]    = sum_m Wf[m,je] * y0[m,b]
  s2_j0 = 0.5*A_j0 ; s2_j1 = 0.5*(S_j1 - A_j1)  (since c1 = 1-c0)
  v = squash(s2)

Inputs are re-laid-out host-side (numpy) into PE-friendly DRAM feeds.
"""

import numpy as np
import ml_dtypes

import concourse.bacc as bacc
import concourse.mybir as mybir
import concourse.tile as tile
from concourse.bass_utils import run_bass_kernel_spmd

# Problem constants (hardcoded per harness contract)
B = 256
NCORES = 8
BC = B // NCORES          # 32 batch per core
N = 6912
D = 8
E = 16
J = 2
M = N * D                 # 55296
JE = J * E                # 32
NT = M // 128             # 432 m-tiles
NG = NT // 4              # 108 groups of 4 (row-packed g matmuls)
CH = M * BC // 128 // 512 # 27 512-col chunks of the [128, 13824] monoliths
FREE = NT * BC            # 13824
EPS = 1e-9

BF16 = mybir.dt.bfloat16
F32 = mybir.dt.float32

_cached = None


def _build_program(level=7):
    nc = bacc.Bacc("TRN2", num_devices=NCORES)

    xt = nc.dram_tensor("xt", [128, FREE], BF16, kind="ExternalInput")
    wf = nc.dram_tensor("wf", [128, FREE], BF16, kind="ExternalInput")
    wft = nc.dram_tensor("wft", [JE, M], BF16, kind="ExternalInput")
    sum16 = nc.dram_tensor("sum16", [128, 16], BF16, kind="ExternalInput")
    rep16 = nc.dram_tensor("rep16", [16, 128], BF16, kind="ExternalInput")
    vout = nc.dram_tensor("vout", [BC, JE], F32, kind="ExternalOutput")

    with tile.TileContext(nc) as tc:
        with (
            tc.tile_pool(name="big", bufs=1) as big,
            tc.tile_pool(name="small", bufs=1) as small,
            tc.tile_pool(name="gevac", bufs=3) as gevac,
            tc.tile_pool(name="ps_acc", bufs=1, space="PSUM") as ps_acc,
            tc.tile_pool(name="ps_g", bufs=3, space="PSUM") as ps_g,
            tc.tile_pool(name="ps_sm", bufs=2, space="PSUM") as ps_sm,
        ):
            XT = big.tile([128, FREE], BF16, tag="XT")
            WF = big.tile([128, FREE], BF16, tag="WF")
            WFT = big.tile([JE, M], BF16, tag="WFT")
            SUM16 = small.tile([128, 16], BF16, tag="SUM16")
            REP16 = small.tile([16, 128], BF16, tag="REP16")

            nc.sync.dma_start(XT[:], xt[:])
            nc.sync.dma_start(WF[:], wf[:])
            nc.sync.dma_start(WFT[:], wft[:])
            nc.sync.dma_start(SUM16[:], sum16[:])
            nc.sync.dma_start(REP16[:], rep16[:])

            # ---- Phase 1: S[b, je] = sum_m x[m,b] * Wf[m,je]  ----
            ps1 = ps_acc.tile([BC, JE], F32, tag="ps1")
            for t in range(NT):
                nc.tensor.matmul(
                    ps1[:],
                    lhsT=XT[:, t * BC:(t + 1) * BC],
                    rhs=WF[:, t * JE:(t + 1) * JE],
                    start=(t == 0),
                    stop=(t == NT - 1),
                )

            # ---- Phase 2: squash -> v1, vtil, vtilT ----
            S = small.tile([BC, JE], F32, tag="S")      # raw sum (kept for s2_j1)
            s = small.tile([BC, JE], F32, tag="s")      # 0.5*S
            sq = small.tile([BC, JE], F32, tag="sq")
            n2 = small.tile([BC, J], F32, tag="n2")
            d1 = small.tile([BC, J], F32, tag="d1")
            r1 = small.tile([BC, J], F32, tag="r1")
            q = small.tile([BC, J], F32, tag="q")
            rq = small.tile([BC, J], F32, tag="rq")
            f = small.tile([BC, J], F32, tag="f")
            vt = small.tile([BC, JE], BF16, tag="vt")
            vtT = small.tile([BC, JE], BF16, tag="vtT")
            VT4 = small.tile([128, BC], BF16, tag="VT4")

            nc.vector.tensor_copy(S[:], ps1[:])
            nc.vector.tensor_scalar_mul(s[:], S[:], 0.5)
            nc.vector.tensor_mul(sq[:], s[:], s[:])
            nc.vector.reduce_sum(
                n2[:], sq.rearrange("p (j e) -> p j e", e=E), axis=mybir.AxisListType.X
            )
            nc.vector.tensor_scalar_add(d1[:], n2[:], 1.0)
            nc.vector.reciprocal(r1[:], d1[:])
            nc.vector.tensor_scalar_add(q[:], n2[:], EPS)
            nc.scalar.activation(q[:], q[:], mybir.ActivationFunctionType.Sqrt)
            nc.vector.reciprocal(rq[:], q[:])
            nc.vector.tensor_mul(f[:], n2[:], r1[:])
            nc.vector.tensor_mul(f[:], f[:], rq[:])
            # vtil = [v1_j0, -v1_j1] in bf16 (fold sign into the factor)
            nc.vector.tensor_scalar_mul(vt[:, 0:E], s[:, 0:E], f[:, 0:1])
            nc.vector.tensor_scalar_mul(f[:, 1:2], f[:, 1:2], -1.0)
            nc.vector.tensor_scalar_mul(vt[:, E:JE], s[:, E:JE], f[:, 1:2])
            # transpose [32,32] block and replicate into 4 partition groups
            nc.vector.transpose(vtT[:], vt[:])

            # ---- Phases 3-6 fused, per 512-col chunk (16 m-tiles) ----
            # g -> T = g*x -> Delta -> sigmoid -> c broadcast -> y0 -> s2 MMs
            ps2 = ps_acc.tile([BC, JE], F32, tag="ps2")
            nch = CH if level >= 3 else 0
            for K in range(nch):
                lo, hi = K * 512, (K + 1) * 512
                psg = ps_g.tile([128, 512], F32, tag="psg")
                for i in range(16):
                    t = 16 * K + i
                    nc.tensor.matmul(
                        psg[:, i * BC:(i + 1) * BC],
                        lhsT=WFT[:, t * 128:(t + 1) * 128],
                        rhs=vtT[:],
                        start=True,
                        stop=True,
                    )
                gbf = gevac.tile([128, 512], BF16, tag="gbf")
                nc.scalar.copy(gbf[:], psg[:])
                tch = gevac.tile([128, 512], BF16, tag="tch")
                nc.vector.tensor_mul(tch[:], gbf[:], XT[:, lo:hi])
                psd = ps_sm.tile([16, 512], F32, tag="psd")
                nc.tensor.matmul(
                    psd[:], lhsT=SUM16[:], rhs=tch[:], start=True, stop=True
                )
                c0c = gevac.tile([16, 512], BF16, tag="c0c")
                nc.scalar.activation(
                    c0c[:], psd[:], mybir.ActivationFunctionType.Sigmoid
                )
                if level >= 5:
                    psc = ps_g.tile([128, 512], F32, tag="psg")
                    nc.tensor.matmul(
                        psc[:], lhsT=REP16[:], rhs=c0c[:], start=True, stop=True
                    )
                    ybf = gevac.tile([128, 512], BF16, tag="ybf")
                    nc.scalar.copy(ybf[:], psc[:])
                    nc.vector.tensor_mul(ybf[:], ybf[:], XT[:, lo:hi])
                    if level >= 6:
                        for i in range(16):
                            t = 16 * K + i
                            nc.tensor.matmul(
                                ps2[:],
                                lhsT=ybf[:, i * BC:(i + 1) * BC],
                                rhs=WF[:, t * JE:(t + 1) * JE],
                                start=(t == 0),
                                stop=(t == NT - 1),
                            )

            # ---- Phase 7: s2, squash, output ----
            s2 = small.tile([BC, JE], F32, tag="s2")
            sq2 = small.tile([BC, JE], F32, tag="sq2")
            n2b = small.tile([BC, J], F32, tag="n2b")
            d1b = small.tile([BC, J], F32, tag="d1b")
            r1b = small.tile([BC, J], F32, tag="r1b")
            qb = small.tile([BC, J], F32, tag="qb")
            rqb = small.tile([BC, J], F32, tag="rqb")
            fb = small.tile([BC, J], F32, tag="fb")
            v2 = small.tile([BC, JE], F32, tag="v2")

            if level < 6:
                sq2 = sq2  # phases 6-7 skipped in bisect builds
            else:
              nc.vector.tensor_copy(s2[:, 0:E], ps2[:, 0:E])
              nc.vector.tensor_sub(s2[:, E:JE], S[:, E:JE], ps2[:, E:JE])
              nc.vector.tensor_scalar_mul(s2[:], s2[:], 0.5)
              nc.vector.tensor_mul(sq2[:], s2[:], s2[:])
              nc.vector.reduce_sum(
                  n2b[:], sq2.rearrange("p (j e) -> p j e", e=E), axis=mybir.AxisListType.X
              )
              nc.vector.tensor_scalar_add(d1b[:], n2b[:], 1.0)
              nc.vector.reciprocal(r1b[:], d1b[:])
              nc.vector.tensor_scalar_add(qb[:], n2b[:], EPS)
              nc.scalar.activation(qb[:], qb[:], mybir.ActivationFunctionType.Sqrt)
              nc.vector.reciprocal(rqb[:], qb[:])
              nc.vector.tensor_mul(fb[:], n2b[:], r1b[:])
              nc.vector.tensor_mul(fb[:], fb[:], rqb[:])
              nc.vector.tensor_scalar_mul(v2[:, 0:E], s2[:, 0:E], fb[:, 0:1])
              nc.vector.tensor_scalar_mul(v2[:, E:JE], s2[:, E:JE], fb[:, 1:2])

            if level >= 6:
                nc.sync.dma_start(vout[:], v2[:])
            else:
                nc.sync.dma_start(vout[:], s[:])

    nc.compile()
    return nc


def _prep_host(x, W):
    """Build per-core DRAM feeds. Returns (in_maps, consts are shared)."""
    bf = ml_dtypes.bfloat16
    # Wf[(n,d), (j,e)] = W[j,n,e,d]
    Wf = np.ascontiguousarray(np.transpose(W, (1, 3, 0, 2)).reshape(M, JE))
    wf_feed = np.ascontiguousarray(
        Wf.reshape(NT, 128, JE).transpose(1, 0, 2).reshape(128, FREE)
    ).astype(bf)
    wft_feed = np.ascontiguousarray(Wf.T).astype(bf)   # [32, M]

    p = np.arange(128)
    sum16_np = (p[:, None] // D == np.arange(16)[None, :]).astype(bf)
    rep16_np = (np.arange(16)[:, None] == p[None, :] // D).astype(bf)

    in_maps = []
    for c in range(NCORES):
        xs = x[c * BC:(c + 1) * BC].reshape(BC, M).T      # [m, b]
        xt_feed = np.ascontiguousarray(
            xs.reshape(NT, 128, BC).transpose(1, 0, 2).reshape(128, FREE)
        ).astype(bf)
        in_maps.append({
            "xt": xt_feed,
            "wf": wf_feed,
            "wft": wft_feed,
            "sum16": sum16_np,
            "rep16": rep16_np,
        })
    return in_maps


def kernel(x, W, level=7):
    global _cached
    x = np.asarray(x, dtype=np.float32)
    W = np.asarray(W, dtype=np.float32)
    if _cached is None:
        _cached = _build_program(level)
    nc = _cached
    in_maps = _prep_host(x, W)
    res = run_bass_kernel_spmd(nc, in_maps, list(range(NCORES)))
    out = np.concatenate(
        [res.results[c]["vout"].reshape(BC, J, E) for c in range(NCORES)], axis=0
    )
    return out.astype(np.float32)


if __name__ == "__main__":
    import sys
    sys.path.insert(0, "/root/problem")
    import reference as ref
    inputs = ref.setup_inputs()
    expected = np.asarray(ref.reference(**inputs))
    actual = kernel(np.asarray(inputs["x"]), np.asarray(inputs["W"]))
    err = np.abs(actual - expected)
    scale = np.abs(expected).max()
    print("absmax err:", err.max(), "scale:", scale, "rel:", err.max() / scale)


# revision 16
# speedup vs baseline: 1.7472x; 1.7472x over previous
"""DigitCapsuleLayer forward (2 routing iterations) on 8 Trainium2 cores.

Pure data-parallel: batch 256 is split 32-per-core. All heavy contractions
run on the PE array in bf16 with f32 PSUM accumulation; routing math is
restructured so u_hat [B,2,6912,16] is never materialized:

  S[b,je]    = sum_m Wf[m,je] * x[m,b]          (m = (n,d) flattened, 55296)
  v1         = squash(0.5*S)
  g[m,b]     = sum_je Wf[m,je] * vtil[je,b]     (vtil = [v1_j0, -v1_j1])
  Delta[n,b] = sum_d g[(n,d),b] * x[(n,d),b]    (block-diag ones matmul)
  c0         = sigmoid(Delta) broadcast over d  (replication matmul)
  y0         = c0 * x
  A[b,je]    = sum_m Wf[m,je] * y0[m,b]
  s2_j0 = 0.5*A_j0 ; s2_j1 = 0.5*(S_j1 - A_j1)  (since c1 = 1-c0)
  v = squash(s2)

Inputs are re-laid-out host-side (numpy) into PE-friendly DRAM feeds.
"""

import numpy as np
import ml_dtypes

import concourse.bacc as bacc
import concourse.mybir as mybir
import concourse.tile as tile
from concourse.bass_utils import run_bass_kernel_spmd

# Problem constants (hardcoded per harness contract)
B = 256
NCORES = 8
BC = B // NCORES          # 32 batch per core
N = 6912
D = 8
E = 16
J = 2
M = N * D                 # 55296
JE = J * E                # 32
NT = M // 128             # 432 m-tiles
NG = NT // 4              # 108 groups of 4 (row-packed g matmuls)
CH = M * BC // 128 // 512 # 27 512-col chunks of the [128, 13824] monoliths
FREE = NT * BC            # 13824
EPS = 1e-9

BF16 = mybir.dt.bfloat16
F32 = mybir.dt.float32

_cached = None


def _build_program(level=7):
    nc = bacc.Bacc("TRN2", num_devices=NCORES)

    xt = nc.dram_tensor("xt", [128, FREE], BF16, kind="ExternalInput")
    wf = nc.dram_tensor("wf", [128, FREE], BF16, kind="ExternalInput")
    wft = nc.dram_tensor("wft", [128, NG * 128], BF16, kind="ExternalInput")
    sumrep = nc.dram_tensor("sumrep", [128, 128], BF16, kind="ExternalInput")
    vout = nc.dram_tensor("vout", [BC, JE], F32, kind="ExternalOutput")

    with tile.TileContext(nc) as tc:
        with (
            tc.tile_pool(name="big", bufs=1) as big,
            tc.tile_pool(name="small", bufs=1) as small,
            tc.tile_pool(name="gevac", bufs=4) as gevac,
            tc.tile_pool(name="ps_acc", bufs=1, space="PSUM") as ps_acc,
            tc.tile_pool(name="ps_g", bufs=3, space="PSUM") as ps_g,
            tc.tile_pool(name="ps_sm", bufs=3, space="PSUM") as ps_sm,
        ):
            XT = big.tile([128, FREE], BF16, tag="XT")
            WF = big.tile([128, FREE], BF16, tag="WF")
            WFT = big.tile([128, NG * 128], BF16, tag="WFT")
            SUMREP = small.tile([128, 128], BF16, tag="SUMREP")

            NSL = 8
            slw = FREE // NSL
            for i in range(NSL):
                nc.sync.dma_start(XT[:, i * slw:(i + 1) * slw], xt[:, i * slw:(i + 1) * slw])
                nc.sync.dma_start(WF[:, i * slw:(i + 1) * slw], wf[:, i * slw:(i + 1) * slw])
            nc.sync.dma_start(SUMREP[:], sumrep[:])

            # ---- Phase 1: S[b, je] = sum_m x[m,b] * Wf[m,je]  ----
            ps1 = ps_acc.tile([BC, JE], F32, tag="ps1")
            for t in range(NT):
                nc.tensor.matmul(
                    ps1[:],
                    lhsT=XT[:, t * BC:(t + 1) * BC],
                    rhs=WF[:, t * JE:(t + 1) * JE],
                    start=(t == 0),
                    stop=(t == NT - 1),
                )

            nc.sync.dma_start(WFT[:], wft[:])

            # ---- Phase 2: squash -> v1, vtil, vtilT ----
            S = small.tile([BC, JE], F32, tag="S")      # raw sum (kept for s2_j1)
            s = small.tile([BC, JE], F32, tag="s")      # 0.5*S
            sq = small.tile([BC, JE], F32, tag="sq")
            n2 = small.tile([BC, J], F32, tag="n2")
            d1 = small.tile([BC, J], F32, tag="d1")
            r1 = small.tile([BC, J], F32, tag="r1")
            q = small.tile([BC, J], F32, tag="q")
            rq = small.tile([BC, J], F32, tag="rq")
            f = small.tile([BC, J], F32, tag="f")
            vt = small.tile([BC, JE], BF16, tag="vt")
            vtT = small.tile([BC, JE], BF16, tag="vtT")
            VT4 = small.tile([128, BC], BF16, tag="VT4")

            nc.vector.tensor_copy(S[:], ps1[:])
            nc.vector.tensor_scalar_mul(s[:], S[:], 0.5)
            nc.vector.tensor_mul(sq[:], s[:], s[:])
            nc.vector.reduce_sum(
                n2[:], sq.rearrange("p (j e) -> p j e", e=E), axis=mybir.AxisListType.X
            )
            nc.vector.tensor_scalar_add(d1[:], n2[:], 1.0)
            nc.vector.reciprocal(r1[:], d1[:])
            nc.vector.tensor_scalar_add(q[:], n2[:], EPS)
            nc.scalar.activation(q[:], q[:], mybir.ActivationFunctionType.Sqrt)
            nc.vector.reciprocal(rq[:], q[:])
            nc.vector.tensor_mul(f[:], n2[:], r1[:])
            nc.vector.tensor_mul(f[:], f[:], rq[:])
            # vtil = [v1_j0, -v1_j1] in bf16 (fold sign into the factor)
            nc.vector.tensor_scalar_mul(vt[:, 0:E], s[:, 0:E], f[:, 0:1])
            nc.vector.tensor_scalar_mul(f[:, 1:2], f[:, 1:2], -1.0)
            nc.vector.tensor_scalar_mul(vt[:, E:JE], s[:, E:JE], f[:, 1:2])
            # transpose [32,32] block and replicate into 4 partition groups
            nc.vector.transpose(vtT[:], vt[:])
            VTBD = small.tile([128, 128], BF16, tag="VTBD")
            nc.vector.memset(VTBD[:], 0.0)
            for a in range(4):
                nc.sync.dma_start(VTBD[32 * a:32 * a + 32, 32 * a:32 * a + 32], vtT[:])

            # ---- Phases 3-6 fused, per 512-col chunk (16 m-tiles) ----
            # g -> T = g*x -> Delta -> sigmoid -> c broadcast -> y0 -> s2 MMs
            ps2 = ps_acc.tile([BC, JE], F32, tag="ps2")
            nch = CH if level >= 3 else 0
            for K in range(nch):
                lo, hi = K * 512, (K + 1) * 512
                psg = ps_g.tile([128, 512], F32, tag="psg")
                for q in range(4):
                    g_idx = 4 * K + q
                    nc.tensor.matmul(
                        psg[:, q * 128:(q + 1) * 128],
                        lhsT=WFT[:, g_idx * 128:(g_idx + 1) * 128],
                        rhs=VTBD[:],
                        start=True,
                        stop=True,
                    )
                gbf = gevac.tile([128, 512], BF16, tag="gbf")
                nc.scalar.copy(gbf[:], psg[:])
                tch = gevac.tile([128, 512], BF16, tag="tch")
                nc.vector.tensor_mul(tch[:], gbf[:], XT[:, lo:hi])
                psd = ps_sm.tile([128, 512], F32, tag="psd")
                nc.tensor.matmul(
                    psd[:], lhsT=SUMREP[:], rhs=tch[:], start=True, stop=True
                )
                if level >= 5:
                    cbf = gevac.tile([128, 512], BF16, tag="cbf")
                    nc.scalar.activation(
                        cbf[:], psd[:], mybir.ActivationFunctionType.Sigmoid
                    )
                    ybf = gevac.tile([128, 512], BF16, tag="ybf")
                    nc.vector.tensor_mul(ybf[:], cbf[:], XT[:, lo:hi])
                    if level >= 6:
                        for i in range(16):
                            t = 16 * K + i
                            nc.tensor.matmul(
                                ps2[:],
                                lhsT=ybf[:, i * BC:(i + 1) * BC],
                                rhs=WF[:, t * JE:(t + 1) * JE],
                                start=(t == 0),
                                stop=(t == NT - 1),
                            )

            # ---- Phase 7: s2, squash, output ----
            s2 = small.tile([BC, JE], F32, tag="s2")
            sq2 = small.tile([BC, JE], F32, tag="sq2")
            n2b = small.tile([BC, J], F32, tag="n2b")
            d1b = small.tile([BC, J], F32, tag="d1b")
            r1b = small.tile([BC, J], F32, tag="r1b")
            qb = small.tile([BC, J], F32, tag="qb")
            rqb = small.tile([BC, J], F32, tag="rqb")
            fb = small.tile([BC, J], F32, tag="fb")
            v2 = small.tile([BC, JE], F32, tag="v2")

            if level < 6:
                sq2 = sq2  # phases 6-7 skipped in bisect builds
            else:
              nc.vector.tensor_copy(s2[:, 0:E], ps2[:, 0:E])
              nc.vector.tensor_sub(s2[:, E:JE], S[:, E:JE], ps2[:, E:JE])
              nc.vector.tensor_scalar_mul(s2[:], s2[:], 0.5)
              nc.vector.tensor_mul(sq2[:], s2[:], s2[:])
              nc.vector.reduce_sum(
                  n2b[:], sq2.rearrange("p (j e) -> p j e", e=E), axis=mybir.AxisListType.X
              )
              nc.vector.tensor_scalar_add(d1b[:], n2b[:], 1.0)
              nc.vector.reciprocal(r1b[:], d1b[:])
              nc.vector.tensor_scalar_add(qb[:], n2b[:], EPS)
              nc.scalar.activation(qb[:], qb[:], mybir.ActivationFunctionType.Sqrt)
              nc.vector.reciprocal(rqb[:], qb[:])
              nc.vector.tensor_mul(fb[:], n2b[:], r1b[:])
              nc.vector.tensor_mul(fb[:], fb[:], rqb[:])
              nc.vector.tensor_scalar_mul(v2[:, 0:E], s2[:, 0:E], fb[:, 0:1])
              nc.vector.tensor_scalar_mul(v2[:, E:JE], s2[:, E:JE], fb[:, 1:2])

            if level >= 6:
                nc.sync.dma_start(vout[:], v2[:])
            else:
                nc.sync.dma_start(vout[:], s[:])

    nc.compile()
    return nc


def _prep_host(x, W):
    """Build per-core DRAM feeds. Returns (in_maps, consts are shared)."""
    bf = ml_dtypes.bfloat16
    # Wf[(n,d), (j,e)] = W[j,n,e,d]
    Wf = np.ascontiguousarray(np.transpose(W, (1, 3, 0, 2)).reshape(M, JE))
    wf_feed = np.ascontiguousarray(
        Wf.reshape(NT, 128, JE).transpose(1, 0, 2).reshape(128, FREE)
    ).astype(bf)
    # 4-stacked WfT groups: group g rows 32a+k hold Wf[m=128*(4g+a)+f, k]
    wft_np = np.empty((NG, 128, 128), dtype=np.float32)
    blocks = Wf.reshape(NT, 128, JE)                    # [432, 128, 32]
    for a in range(4):
        wft_np[:, 32 * a:32 * a + 32, :] = blocks[a::4].transpose(0, 2, 1)
    wft_feed = np.ascontiguousarray(
        wft_np.transpose(1, 0, 2).reshape(128, NG * 128)
    ).astype(bf)

    p = np.arange(128)
    sumrep_np = (p[:, None] // D == p[None, :] // D).astype(bf)

    in_maps = []
    for c in range(NCORES):
        xs = x[c * BC:(c + 1) * BC].reshape(BC, M).T      # [m, b]
        xt_feed = np.ascontiguousarray(
            xs.reshape(NT, 128, BC).transpose(1, 0, 2).reshape(128, FREE)
        ).astype(bf)
        in_maps.append({
            "xt": xt_feed,
            "wf": wf_feed,
            "wft": wft_feed,
            "sumrep": sumrep_np,
        })
    return in_maps


def kernel(x, W, level=7):
    global _cached
    x = np.asarray(x, dtype=np.float32)
    W = np.asarray(W, dtype=np.float32)
    if _cached is None:
        _cached = _build_program(level)
    nc = _cached
    in_maps = _prep_host(x, W)
    res = run_bass_kernel_spmd(nc, in_maps, list(range(NCORES)))
    out = np.concatenate(
        [res.results[c]["vout"].reshape(BC, J, E) for c in range(NCORES)], axis=0
    )
    return out.astype(np.float32)


if __name__ == "__main__":
    import sys
    sys.path.insert(0, "/root/problem")
    import reference as ref
    inputs = ref.setup_inputs()
    expected = np.asarray(ref.reference(**inputs))
    actual = kernel(np.asarray(inputs["x"]), np.asarray(inputs["W"]))
    err = np.abs(actual - expected)
    scale = np.abs(expected).max()
    print("absmax err:", err.max(), "scale:", scale, "rel:", err.max() / scale)
